# revision 43
# baseline (speedup 1.0000x reference)
"""Multi-head attention (softmax over query axis) on 8 Trainium2 cores.

Problem: nn_MultiHeadAttention_3899830305178
  B=2, S=2048, D_MODEL=1024, HEADS=16, D_K=64, fp32 IO.
  reference:
    q = (query @ Wq + bq), k = ..., v = ...        [b, s, h, dk]
    scores = einsum('bihd,bjhd->bijh', q, k) / 8
    attn = softmax(scores, axis=1)                 # over QUERY axis i (quirk)
    x = einsum('bijh,bjhd->bihd', attn, v)         [b, s, h*dk]
    out = x @ Wo + bo

Sharding: data-parallel over batch (2) x tensor-parallel over heads (4 groups
of 4 heads) = 8 cores. Each core computes a partial output
O_part = x_local @ Wo[rows of its heads]; the host sums the 4 partials per
batch (row-parallel unshard) -- bo is added on-device by the g==0 core.

Per-core kernel math (host passes query/key/value pre-transposed so the
projections contract over the model dim on partitions):
  qT[d', i] = Wq_s.T @ queryT      (d' = 4 local heads x 64 = 256)
  kT[d', j] = Wk_s.T @ keyT
  vT[d', j] = Wv_s.T @ valueT (+bv), then bf16 DMA-transpose -> v[j, d']
  per head h:  sT[j, i] = kT_h.T @ qT_h / 8  (softmax over i == free axis)
               eT = exp(sT), rowsum over i fused via ACT accum_out
               v_h_scaled[j, :] = v_h[j, :] / rowsum[j]   <- softmax divisor
               xT_h[d, i] = v_h_scaled.T @ eT             (contracts over j)
  O_part[i, n] = xT.T @ Wo_s (+ bo on one core per batch)

Projection inputs/weights are bf16; attention and the output projection run
in float32r (TF32, fp32 accumulate) with fp32 softmax statistics. Measured
end-to-end relative error vs the fp32 reference is ~4e-3 (CoreSim).
"""

import numpy as np

import concourse.bass as bass
import concourse.mybir as mybir
import concourse.tile as tile
from concourse.bass_utils import run_bass_kernel_spmd

# problem shape (hardcoded per contract)
B, S, DM, H, DK = 2, 2048, 1024, 16, 64
N_CORES = 8
GROUPS = 4              # head groups (tensor-parallel)
HL = H // GROUPS        # 4 local heads per core
DL = HL * DK            # 256 local concat width
P = 128
SJ = S // P             # 16 strips of 128 along j (keys) and i (out rows)
MT = DM // P            # 8 contraction tiles for projections
DPT = DL // P           # 2 partition tiles of the local concat dim
SCALE = 1.0 / 8.0       # 1/sqrt(DK)

f32 = mybir.dt.float32
f32r = mybir.dt.float32r
bf16 = mybir.dt.bfloat16
AF = mybir.ActivationFunctionType

# Projection stage (inputs + projection weights) in bf16: halves the input
# DMA (the critical-path prefix) at ~2e-3 relative error. Attention and
# output projection stay TF32.
PROJ_BF16 = True
PROJ_DT = bf16 if PROJ_BF16 else f32r

_PROGRAM = None


def _split_excess_waits(nc, max_waits=1):
    """walrus in this container rejects >1 semaphore wait per instruction
    (e.g. the Tile kernel-tail Drain); move extras onto same-engine NOPs."""
    n_split = 0
    for f in nc.m.functions:
        for blk in f.blocks:
            new_insts = []
            for inst in blk.instructions:
                si = getattr(inst, "sync_info", None)
                if si is not None and si.on_wait and len(si.on_wait) > max_waits:
                    waits = list(si.on_wait)
                    extra, keep = waits[:-max_waits], waits[-max_waits:]
                    for i in range(0, len(extra), max_waits):
                        chunk = extra[i:i + max_waits]
                        nop = mybir.InstNoOp(
                            name=f"{inst.name}-ws{n_split}-{i}",
                            engine=inst.engine,
                            sync_info=mybir.SyncInfo(on_wait=chunk, on_update=[]),
                            bass_nofuse=True,
                        )
                        new_insts.append(nop)
                    si.on_wait = keep
                    n_split += 1
                new_insts.append(inst)
            blk.instructions[:] = new_insts
    return n_split


def build_program(split_waits=True):
    nc = bass.Bass("TRN2", target_bir_lowering=False, debug=False)

    qT_in = nc.dram_tensor("qT_in", [DM, S], PROJ_DT, kind="ExternalInput")
    kT_in = nc.dram_tensor("kT_in", [DM, S], PROJ_DT, kind="ExternalInput")
    vT_in = nc.dram_tensor("vT_in", [DM, S], PROJ_DT, kind="ExternalInput")
    wq_d = nc.dram_tensor("wq", [DM, DL], PROJ_DT, kind="ExternalInput")
    wk_d = nc.dram_tensor("wk", [DM, DL], PROJ_DT, kind="ExternalInput")
    wv_d = nc.dram_tensor("wv", [DM, DL], PROJ_DT, kind="ExternalInput")
    wo_d = nc.dram_tensor("wo", [DL, DM], f32r, kind="ExternalInput")
    bq_d = nc.dram_tensor("bq", [DL, 1], f32, kind="ExternalInput")
    bk_d = nc.dram_tensor("bk", [DL, 1], f32, kind="ExternalInput")
    bv_d = nc.dram_tensor("bv", [DL, 1], f32, kind="ExternalInput")
    bo_d = nc.dram_tensor("bo_bc", [P, DM], f32, kind="ExternalInput")
    O_d = nc.dram_tensor("O", [S, DM], f32, kind="ExternalOutput")

    with tile.TileContext(nc) as tc:
        with (
            tc.tile_pool(name="const", bufs=1) as const,
            tc.tile_pool(name="persist", bufs=1) as sb,
            tc.tile_pool(name="stat", bufs=6) as stat,
            tc.tile_pool(name="outp", bufs=3) as outp,
            tc.tile_pool(name="inp", bufs=2) as inp,
            tc.tile_pool(name="vtp", bufs=1) as vtp,
            tc.tile_pool(name="attp", bufs=16) as attp,
            tc.tile_pool(name="pps", bufs=2, space="PSUM") as pps,
            tc.tile_pool(name="ppx", bufs=1, space="PSUM") as ppx,
        ):
            # ---------------- constants ----------------
            # One DMA per weight: DRAM [(t p), c] -> SBUF [p, (t c)] so the
            # m-th 128-row block lands at free offset m*DL.
            def load_w(dram, nm, dt_, cols):
                t = const.tile([P, MT * cols], dt_, name=nm, tag=nm)
                nc.sync.dma_start(
                    t[:].rearrange("p (t c) -> p t c", t=MT),
                    dram.ap().rearrange("(t p) c -> p t c", p=P))
                return t

            wq_sb = load_w(wq_d, "wq", PROJ_DT, DL)   # [128, 8*256]
            wk_sb = load_w(wk_d, "wk", PROJ_DT, DL)
            wv_sb = load_w(wv_d, "wv", PROJ_DT, DL)
            wo_sb = const.tile([P, DPT * DM], f32r, name="wo", tag="wo")
            nc.sync.dma_start(
                wo_sb[:].rearrange("p (t c) -> p t c", t=DPT),
                wo_d.ap().rearrange("(t p) c -> p t c", p=P))
            bq_sb = const.tile([P, DPT], f32, name="bq", tag="bq")
            nc.sync.dma_start(
                bq_sb[:].rearrange("p (t c) -> p t c", t=DPT),
                bq_d.ap().rearrange("(t p) c -> p t c", p=P))
            bk_sb = const.tile([P, DPT], f32, name="bk", tag="bk")
            nc.sync.dma_start(
                bk_sb[:].rearrange("p (t c) -> p t c", t=DPT),
                bk_d.ap().rearrange("(t p) c -> p t c", p=P))
            bv_sb = const.tile([P, DPT], f32, name="bv", tag="bv")
            nc.sync.dma_start(
                bv_sb[:].rearrange("p (t c) -> p t c", t=DPT),
                bv_d.ap().rearrange("(t p) c -> p t c", p=P))
            bo_sb = const.tile([P, DM], f32, name="bo", tag="bo")
            nc.sync.dma_start(bo_sb[:], bo_d.ap())

            def w_slice(w, m, dp):
                return w[:, m * DL + dp * P:m * DL + (dp + 1) * P]

            # ---------------- persistent activations ----------------
            # qT split by i-half, kT by i-quarter (j-group): finer tiles give
            # the scheduler finer dependencies, so scores start before the
            # whole projection finishes.
            qT_sb = [[sb.tile([P, 1024], f32r, name=f"qT{dp}_{ih}",
                              tag=f"qT{dp}_{ih}") for ih in range(2)]
                     for dp in range(DPT)]
            kT_sb = [[sb.tile([P, 512], f32r, name=f"kT{dp}_{jg}",
                              tag=f"kT{dp}_{jg}") for jg in range(4)]
                     for dp in range(DPT)]
            # v packed per j-group of 4: v4_sb[jg][p, jj*DL + d'] holds
            # v[jg*512 + jj*128 + p, d']
            v4_sb = [sb.tile([P, 4 * DL], bf16, name=f"v{jg}", tag=f"v{jg}")
                     for jg in range(4)]
            xT_sb = [sb.tile([P, S], f32r, name=f"xT{hp}", tag=f"xT{hp}")
                     for hp in range(DPT)]

            # ---------------- projections ----------------
            # dst[d', i] = W.T @ inT ; contraction over m on partitions.
            # Projection psums share the scores pool ("ps" tag): both dp
            # halves of one i-chunk go into one [P, 1024] psum tile.
            vT_sb = [vtp.tile([P, S], bf16, name=f"vT{dp}", tag=f"vT{dp}")
                     for dp in range(DPT)]

            def load_in_chunk(win, nm, i4):
                # one DMA: all 8 m-blocks of columns [i0, i0+512)
                t = inp.tile([P, MT * 512], PROJ_DT, name=f"{nm}in{i4}",
                             tag="pin")
                src = win.ap().rearrange("(t p) c -> p t c", p=P)
                nc.sync.dma_start(
                    t[:].rearrange("p (t c) -> p t c", t=MT),
                    src[:, :, i4 * 512:(i4 + 1) * 512])
                return t

            for i4 in range(4):
                for nm, win, w_sb, b_sb in (
                    ("q", qT_in, wq_sb, bq_sb),
                    ("k", kT_in, wk_sb, bk_sb),
                ):
                    ch = load_in_chunk(win, nm, i4)
                    ps = ppx.tile([P, 1024], f32, name=f"ps{nm}{i4}",
                                  tag="px")
                    for dp in range(DPT):
                        for m in range(MT):
                            nc.tensor.matmul(
                                ps[:, dp * 512:(dp + 1) * 512],
                                w_slice(w_sb, m, dp),
                                ch[:, m * 512:(m + 1) * 512],
                                start=(m == 0), stop=(m == MT - 1))
                    for dp in range(DPT):
                        if nm == "q":
                            dst = qT_sb[dp][i4 // 2][:, (i4 % 2) * 512:
                                                     (i4 % 2) * 512 + 512]
                        else:
                            dst = kT_sb[dp][i4][:]
                        nc.vector.tensor_scalar_add(
                            dst, ps[:, dp * 512:(dp + 1) * 512],
                            b_sb[:, dp:dp + 1])

            # vT[d', j] = Wv.T @ valueT (bias folded in, bf16 out), then one
            # SBUF->SBUF bf16 DMA-transpose per (i4, dp) covering 4 j-tiles,
            # dispatched on the ACT HWDGE queue to keep SP free for inputs.
            for i4 in range(4):
                i0 = i4 * 512
                ch = load_in_chunk(vT_in, "v", i4)
                ps = ppx.tile([P, 1024], f32, name=f"psvt{i4}", tag="px")
                for dp in range(DPT):
                    for m in range(MT):
                        nc.tensor.matmul(
                            ps[:, dp * 512:(dp + 1) * 512],
                            w_slice(wv_sb, m, dp),
                            ch[:, m * 512:(m + 1) * 512],
                            start=(m == 0), stop=(m == MT - 1))
                for dp in range(DPT):
                    nc.vector.tensor_scalar_add(
                        vT_sb[dp][:, i0:i0 + 512],
                        ps[:, dp * 512:(dp + 1) * 512], bv_sb[:, dp:dp + 1])
                for dp in range(DPT):
                    out_view = v4_sb[i4][:].rearrange(
                        "p (j c) -> p j c", j=4)[:, :,
                                                 dp * P:(dp + 1) * P]
                    nc.scalar.dma_start(
                        out_view, vT_sb[dp][:, i0:i0 + 512], transpose=True)

            # ---------------- attention (per head) ----------------
            # Emitted after the vT/transpose section but priority-shifted so
            # the scheduler starts scores/exp as soon as q/k are projected,
            # filling PE gaps with the vT projection work (deps still hold).
            prio = tc.high_priority()
            prio.__enter__()
            for h in range(HL):
                hp, hh = divmod(h, 2)
                base = hh * 64
                xps = ppx.tile([64, S], f32, name=f"xps{h}", tag="px")
                for j in range(SJ):
                    jg, jr = divmod(j, 4)
                    ah = []
                    rs_halves = []
                    for ih in range(2):
                        a = attp.tile([P, 1024], f32r,
                                      name=f"att{h}_{j}_{ih}", tag="att")
                        ps = pps.tile([P, 1024], f32,
                                      name=f"pss{h}_{j}_{ih}", tag="ps")
                        for i5 in range(2):
                            io = i5 * 512
                            nc.tensor.matmul(
                                ps[:, io:io + 512],
                                kT_sb[hp][jg][base:base + 64,
                                              jr * P:(jr + 1) * P],
                                qT_sb[hp][ih][base:base + 64, io:io + 512],
                                start=True, stop=True)
                        rsh = stat.tile([P, 1], f32, name=f"rsh{h}_{j}_{ih}",
                                        tag="rsh")
                        nc.scalar.activation(
                            a[:], ps[:], AF.Exp, scale=SCALE,
                            accum_out=rsh[:])
                        ah.append(a)
                        rs_halves.append(rsh)
                    rs = stat.tile([P, 1], f32, name=f"rs{h}_{j}", tag="rs")
                    nc.vector.tensor_add(rs[:], rs_halves[0][:],
                                         rs_halves[1][:])
                    rc = stat.tile([P, 1], f32, name=f"rc{h}_{j}", tag="rc")
                    nc.vector.reciprocal(rc[:], rs[:])
                    vsc = attp.tile([P, 64], f32r, name=f"vsc{h}_{j}",
                                    tag="vsc", bufs=3)
                    nc.vector.tensor_scalar_mul(
                        vsc[:],
                        v4_sb[jg][:, jr * DL + h * 64:jr * DL + (h + 1) * 64],
                        rc[:])
                    for i5 in range(4):
                        io = (i5 % 2) * 512
                        nc.tensor.matmul(
                            xps[:, i5 * 512:(i5 + 1) * 512], vsc[:],
                            ah[i5 // 2][:, io:io + 512],
                            start=(j == 0), stop=(j == SJ - 1),
                            skip_group_check=True)
                nc.vector.tensor_copy(xT_sb[hp][base:base + 64, :], xps[:])
            prio.__exit__(None, None, None)

            # ---------------- output projection ----------------
            for jt in range(SJ):
                ot = outp.tile([P, DM], f32, name=f"ot{jt}", tag="ot")
                for n5 in range(2):
                    no = n5 * 512
                    ps = pps.tile([P, 512], f32, name=f"pso{jt}_{n5}",
                                  tag="ps")
                    for cpt in range(DPT):
                        nc.tensor.matmul(
                            ps[:], xT_sb[cpt][:, jt * P:(jt + 1) * P],
                            wo_sb[:, cpt * DM + no:cpt * DM + no + 512],
                            start=(cpt == 0), stop=(cpt == DPT - 1))
                    nc.vector.tensor_add(ot[:, no:no + 512], ps[:],
                                         bo_sb[:, no:no + 512])
                nc.sync.dma_start(O_d.ap()[jt * P:(jt + 1) * P, :], ot[:])

    if split_waits:
        _split_excess_waits(nc)
    return nc


def _get_program():
    global _PROGRAM
    if _PROGRAM is None:
        _PROGRAM = build_program()
    return _PROGRAM


def _tf32(x):
    """Round fp32 -> TF32 (10-bit mantissa), round-to-nearest-even."""
    x = np.ascontiguousarray(np.asarray(x, dtype=np.float32))
    u = x.view(np.uint32)
    r = ((u >> 13) & 1).astype(np.uint32)
    u2 = ((u + np.uint32(0x0FFF) + r) & np.uint32(0xFFFFE000))
    return u2.view(np.float32)


def shard_inputs(inputs):
    """FULL inputs -> per-core in_maps (list of 8 dicts)."""
    q = np.asarray(inputs["query"], dtype=np.float32)
    k = np.asarray(inputs["key"], dtype=np.float32)
    v = np.asarray(inputs["value"], dtype=np.float32)
    Wq = np.asarray(inputs["Wq"], dtype=np.float32)
    Wk = np.asarray(inputs["Wk"], dtype=np.float32)
    Wv = np.asarray(inputs["Wv"], dtype=np.float32)
    Wo = np.asarray(inputs["Wo"], dtype=np.float32)
    bq = np.asarray(inputs["bq"], dtype=np.float32)
    bk = np.asarray(inputs["bk"], dtype=np.float32)
    bv = np.asarray(inputs["bv"], dtype=np.float32)
    bo = np.asarray(inputs["bo"], dtype=np.float32)

    if PROJ_BF16:
        import ml_dtypes

        def _proj_cast(x):
            return np.ascontiguousarray(np.asarray(x, np.float32)).astype(
                ml_dtypes.bfloat16)
    else:
        _proj_cast = _tf32

    qT = [_proj_cast(q[b].T) for b in range(B)]
    kT = [_proj_cast(k[b].T) for b in range(B)]
    vT = [_proj_cast(v[b].T) for b in range(B)]

    in_maps = []
    for c in range(N_CORES):
        b, g = c // GROUPS, c % GROUPS
        sl = slice(g * DL, (g + 1) * DL)
        bo_bc = (np.ascontiguousarray(np.broadcast_to(bo, (P, DM)))
                 if g == 0 else np.zeros((P, DM), np.float32))
        in_maps.append({
            "qT_in": qT[b],
            "kT_in": kT[b],
            "vT_in": vT[b],
            "wq": _proj_cast(Wq[:, sl]),
            "wk": _proj_cast(Wk[:, sl]),
            "wv": _proj_cast(Wv[:, sl]),
            "wo": _tf32(Wo[sl, :]),
            "bq": np.ascontiguousarray(bq[sl].reshape(DL, 1)),
            "bk": np.ascontiguousarray(bk[sl].reshape(DL, 1)),
            "bv": np.ascontiguousarray(bv[sl].reshape(DL, 1)),
            "bo_bc": bo_bc,
        })
    return in_maps


def unshard_output(results):
    """results: list of 8 dicts with 'O' [S, DM] -> full [B, S, DM]."""
    out = np.zeros((B, S, DM), np.float32)
    for c in range(N_CORES):
        out[c // GROUPS] += results[c]["O"]
    return out


def kernel(**inputs):
    nc = _get_program()
    in_maps = shard_inputs(inputs)
    res = run_bass_kernel_spmd(nc, in_maps, core_ids=list(range(N_CORES)))
    return unshard_output(res.results)
